# revision 19
# baseline (speedup 1.0000x reference)
"""GroupLinear (MoE routing) Trainium2 kernel.

Problem: x [8,2048,1024] f32, group_by [8,2048] int32 in [0,8),
W [8, 1024*1024] f32 (row g -> (dout,din) weight), b [8,1024] f32.
out[b,s,:] = W[g].reshape(1024,1024) @ x[b,s,:] + b[g],  g = group_by[b,s].

Strategy: expert-parallel over 8 NeuronCores. Core g gets every token
routed to group g (host-side dispatch), its own weight (pre-transposed to
[din, dout] so the contraction dim lands on SBUF partitions), and bias.
On device each core runs a single dense GEMM  Y[C,1024] = Xg @ Wg^T + bg,
with PSUM accumulating over the 8 contraction (k) chunks. Tokens beyond
the C capacity (none at seed-0 counts for C>=2088) are finished on the
host. Host scatters the per-core results back to token order.

v2: consecutive matmuls are arranged to share the stationary (lhsT)
operand and a post-emit pass drops the redundant InstLdweights (the PE
array keeps the stationary operand resident between matmuls). Two
layouts:

- 'xs' (X-stationary): stationary = X^T token tile, moving = W columns;
  psum is [token, dout]; k-outer over both 512-wide output halves gives
  2 matmuls per weight load -> 128 LDWEIGHTS per iteration at C=2048.
- 'ws' (W-stationary, default): stationary = W^T tile [128d,128o],
  moving = X^T tokens; psum is [dout_block, token]; for each (o-block,
  k-chunk) the four 512-token moving blocks share one weight load ->
  64 LDWEIGHTS per iteration. Bias becomes per-partition ([128,1]
  broadcast on the DVE evacuation); output is the transposed [DOUT, C]
  (host undoes).

Output is stored f16 (tolerance 2e-2; f16 adds ~2e-4), halving the
store DMA bytes.
"""

import numpy as np
from contextlib import ExitStack

import concourse.bass as bass
import concourse.mybir as mybir
import concourse.tile as tile
from concourse import bacc
from concourse.bass_utils import run_bass_kernel_spmd

B, S, DIN, DOUT, G = 8, 2048, 1024, 1024, 8
P = 128
KC = DIN // P     # 8 contraction chunks
OH = DOUT // 512  # 2 moving halves (moving free dim caps at one PSUM bank)

# f16 operands: FWL halves the LDWEIGHTS cost vs f32; rel err ~2.5e-4.
# C=2048 keeps 16 token blocks; the ~72 tokens above capacity (seed-0 max
# group count 2088) finish on the host.
C_DEFAULT = 2048          # per-core token capacity (16 * 128)
DT_DEFAULT = "f16"        # matmul operand dtype: f32r | f16 | bf16
ODT_DEFAULT = "f16"       # DRAM output dtype: f32 | f16
LAYOUT_DEFAULT = "ws"     # ws (W-stationary) | xs (X-stationary)
OB = DOUT // P            # 8 output blocks of 128 (ws layout)

_cache = {}


def _emit(ctx, tc, y, xt, wt, bias, C, mdt, odt, reps=1):
    nc = tc.nc
    f32 = mybir.dt.float32
    TB = C // P

    singles = ctx.enter_context(tc.tile_pool(name="singles", bufs=1))
    xpool = ctx.enter_context(tc.tile_pool(name="xpool", bufs=8))
    opool = ctx.enter_context(tc.tile_pool(name="opool", bufs=8))
    psum = ctx.enter_context(tc.tile_pool(name="psum", bufs=8, space="PSUM"))

    xt_r = xt.rearrange("(k p) t -> p k t", p=P)
    PH0 = 4  # t-blocks covered by the k-outer warmup phase (PH0*OH psum banks)

    def load_xt(tb):
        xt_tile = xpool.tile([P, KC, P], mdt, name="xt_tile", tag="xt_tile")
        # ACT HWDGE ring: runs concurrently with the weight stream on SP's
        nc.scalar.dma_start(out=xt_tile, in_=xt_r[:, :, tb * P:(tb + 1) * P])
        return xt_tile

    # Prefetch the warmup blocks ahead of the bulk weight stream.
    prefetched = {tb: load_xt(tb) for tb in range(PH0)}

    # Whole weight table resident in SBUF, loaded one DMA per k-chunk so the
    # first matmuls only wait for chunk 0 (split in half so the very first
    # matmul only waits for its own 512-wide slice).
    wt_sb = singles.tile([P, KC, DOUT], mdt)
    wt_r = wt.rearrange("(k p) o -> p k o", p=P)
    nc.sync.dma_start(out=wt_sb[:, 0, 0:512], in_=wt_r[:, 0, 0:512])
    nc.sync.dma_start(out=wt_sb[:, 0, 512:DOUT], in_=wt_r[:, 0, 512:DOUT])
    for k in range(1, KC):
        nc.sync.dma_start(out=wt_sb[:, k, :], in_=wt_r[:, k, :])
    bias_sb = singles.tile([P, DOUT], f32)
    nc.sync.dma_start(out=bias_sb, in_=bias)

    def emit_out(ps, tb, oh):
        ot = opool.tile([P, 512], odt, name="ot", tag="ot")
        nc.vector.tensor_add(out=ot, in0=ps, in1=bias_sb[:, oh * 512:(oh + 1) * 512])
        # SP HWDGE ring: idle after the head weight stream, and ~1.4us less
        # completion latency than the gpsimd SWDGE path (matters for the tail)
        nc.sync.dma_start(out=y[tb * P:(tb + 1) * P, oh * 512:(oh + 1) * 512], in_=ot)

    def mm(ps, xt_tile, k, oh):
        nc.tensor.matmul(
            ps,
            lhsT=xt_tile[:, k, :],
            rhs=wt_sb[:, k, oh * 512:(oh + 1) * 512],
            start=(k == 0),
            stop=(k == KC - 1),
        )

    for _rep in range(reps):
        if _rep == 0:
            # Phase 0: k-outer across PH0 blocks x both halves so the PE
            # consumes each weight chunk the moment its DMA lands instead of
            # stalling a full psum group on the whole weight stream. Both
            # halves of one block are adjacent so they share the stationary
            # X tile (the LDWEIGHTS dedup pass drops the second load).
            tiles0 = [prefetched.pop(tb) for tb in range(PH0)]
            ps0 = [psum.tile([P, 512], f32, name="ps", tag="ps")
                   for _ in range(PH0 * OH)]
            for k in range(KC):
                for i in range(PH0 * OH):
                    tb, oh = divmod(i, OH)
                    mm(ps0[i], tiles0[tb], k, oh)
            for i in range(PH0 * OH):
                tb, oh = divmod(i, OH)
                emit_out(ps0[i], tb, oh)
        start_tb = PH0 if _rep == 0 else 0
        for tb in range(start_tb, TB):
            xt_tile = load_xt(tb)
            # k-outer, halves inner: the two 512-wide matmuls of each
            # k-chunk share one stationary load.
            pss = [psum.tile([P, 512], f32, name="ps", tag="ps")
                   for _ in range(OH)]
            for k in range(KC):
                for oh in range(OH):
                    mm(pss[oh], xt_tile, k, oh)
            for oh in range(OH):
                emit_out(pss[oh], tb, oh)


def _emit_ws(ctx, tc, y, xt, wt, bias, C, mdt, odt, reps=1):
    """W-stationary layout. y is [DOUT, C] (transposed); psum tiles are
    [128 dout, 512 tokens]; for each (o-block, k-chunk) one stationary
    W tile serves all C/512 moving token blocks."""
    nc = tc.nc
    f32 = mybir.dt.float32
    TBS = C // 512  # 512-token moving blocks

    singles = ctx.enter_context(tc.tile_pool(name="singles", bufs=1))
    xpool = ctx.enter_context(tc.tile_pool(name="xpool", bufs=2))
    opool = ctx.enter_context(tc.tile_pool(name="opool", bufs=8))
    psum = ctx.enter_context(tc.tile_pool(name="psum", bufs=8, space="PSUM"))

    xt_r = xt.rearrange("(k p) t -> p k t", p=P)

    # Weights arrive host-pre-shuffled as [OB, P, KC, P] — exactly the SBUF
    # layout per o-block — so each per-o-block DMA is one fully contiguous
    # 256KB read with 2KB per-partition lines (the naive [DIN, DOUT] slice
    # would be 256B-line gather, ~2x slower and it gates the first matmul).
    # First block on the fast SP HWDGE ring; the rest on the gpsimd SWDGE
    # ring behind it.
    wt_sb = singles.tile([P, OB, KC, P], mdt)
    nc.sync.dma_start(out=wt_sb[:, 0], in_=wt[0])
    for ob in range(1, OB):
        nc.gpsimd.dma_start(out=wt_sb[:, ob], in_=wt[ob])
    bias_sb = singles.tile([P, OB], f32)
    nc.gpsimd.dma_start(out=bias_sb, in_=bias)

    def load_xt():
        # Split the 8 k-chunks across both HWDGE rings so the head of a
        # cold run gets ~2x the delivery rate (the first o-block is
        # DMA-paced). k=0 is further split in half so the very first
        # matmul gates on 256KB instead of 512KB.
        xt_tile = xpool.tile([P, KC, C], mdt, name="xt_tile", tag="xt_tile")
        h = C // 2
        nc.scalar.dma_start(out=xt_tile[:, 0, 0:h], in_=xt_r[:, 0, 0:h])
        nc.scalar.dma_start(out=xt_tile[:, 0, h:C], in_=xt_r[:, 0, h:C])
        for k in range(1, KC):
            eng = nc.scalar if k % 2 == 0 else nc.sync
            eng.dma_start(out=xt_tile[:, k, :], in_=xt_r[:, k, :])
        return xt_tile

    for _rep in range(reps):
        xt_tile = load_xt()
        for ob in range(OB):
            pss = [psum.tile([P, 512], f32, name="ps", tag="ps")
                   for _ in range(TBS)]
            for k in range(KC):
                for s in range(TBS):
                    nc.tensor.matmul(
                        pss[s],
                        lhsT=wt_sb[:, ob, k, :],
                        rhs=xt_tile[:, k, s * 512:(s + 1) * 512],
                        start=(k == 0),
                        stop=(k == KC - 1),
                    )
            for s in range(TBS):
                ot = opool.tile([P, 512], odt, name="ot", tag="ot")
                # DVE: psum read + per-partition bias broadcast + f32->f16.
                # (An ACT-engine activation(Identity, bias) variant hit
                # NRT_EXEC_UNIT_UNRECOVERABLE on hardware when combined with
                # full LDWEIGHTS elision; the PE->DVE evacuation path is the
                # production-standard one and runs clean.)
                nc.vector.tensor_scalar_add(
                    out=ot, in0=pss[s], scalar1=bias_sb[:, ob:ob + 1])
                # Last block's stores ride the SP HWDGE ring (~1.4us less
                # completion latency than SWDGE -> shorter kernel tail).
                eng = nc.sync if ob == OB - 1 else nc.gpsimd
                eng.dma_start(
                    out=y[ob * P:(ob + 1) * P, s * 512:(s + 1) * 512], in_=ot)


def _elide_redundant_ldweights(nc):
    """Drop InstLdweights whose access pattern equals the immediately
    preceding PE weight load (no intervening PE weight change): the PE
    array keeps the stationary operand resident across matmuls, so the
    reload is pure overhead. Only fires for loads nothing depends on by
    name and whose SBUF source is never rewritten (weights/activations
    here are write-once per tile-buffer generation; tiles in different
    pool slots have distinct memrefs, so the AP key also changes when a
    buffer is reused for a new token block)."""
    fn = nc.m.functions[0]
    referenced = set()
    for blk in fn.blocks:
        for inst in blk.instructions:
            for dep, _ in inst.dependency_edges():
                referenced.add(dep)
    removed = 0
    for blk in fn.blocks:
        kept = []
        last_key = None
        for inst in blk.instructions:
            cn = type(inst).__name__
            if cn == "InstLdweights":
                ap = inst.ins[0]
                key = (ap.memref, ap.offset, str(ap.ap), str(ap.dtype))
                if key == last_key and inst.name not in referenced:
                    removed += 1
                    continue
                last_key = key
            elif cn == "InstMatmult":
                if inst.is_transpose:
                    last_key = None  # transpose mode loads the array itself
            elif cn in ("InstEventSemaphore", "InstNop", "InstDrain"):
                pass  # sequencer-only: cannot touch the PE array
            elif getattr(inst, "engine", None) == mybir.EngineType.PE:
                last_key = None  # unknown PE instruction: be conservative
            kept.append(inst)
        blk.instructions[:] = kept
    return removed


def _build(reps=1, C=C_DEFAULT, dt=DT_DEFAULT, odt=ODT_DEFAULT,
           layout=LAYOUT_DEFAULT, elide=True):
    key = (reps, C, dt, odt, layout, elide)
    if key in _cache:
        return _cache[key]
    nc = bacc.Bacc("TRN2", target_bir_lowering=False, debug=False,
                   enable_asserts=False, num_devices=G)
    f32 = mybir.dt.float32
    mdt = {"f32r": mybir.dt.float32r, "f16": mybir.dt.float16,
           "bf16": mybir.dt.bfloat16}[dt]
    odt_b = {"f32": mybir.dt.float32, "f16": mybir.dt.float16}[odt]
    # For f32r the DRAM inputs carry the same bits as f32; declaring them
    # f32r end-to-end keeps the BIR verifier's rounding rule satisfied.
    in_dt = mdt if dt != "f32r" else mybir.dt.float32r
    xt = nc.dram_tensor("xt", [DIN, C], in_dt, kind="ExternalInput").ap()
    if layout == "ws":
        # W host-pre-shuffled to the SBUF layout, o-block-major (see _emit_ws)
        wt = nc.dram_tensor("wt", [OB, P, KC, P], in_dt,
                            kind="ExternalInput").ap()
        bias = nc.dram_tensor("bias", [P, OB], f32, kind="ExternalInput").ap()
        y = nc.dram_tensor("y", [DOUT, C], odt_b, kind="ExternalOutput").ap()
    else:
        wt = nc.dram_tensor("wt", [DIN, DOUT], in_dt, kind="ExternalInput").ap()
        bias = nc.dram_tensor("bias", [P, DOUT], f32, kind="ExternalInput").ap()
        y = nc.dram_tensor("y", [C, DOUT], odt_b, kind="ExternalOutput").ap()
    emit = _emit_ws if layout == "ws" else _emit
    with tile.TileContext(nc) as tc, ExitStack() as ctx:
        emit(ctx, tc, y, xt, wt, bias, C, mdt, odt_b, reps=reps)
    if elide:
        nc._n_elided = _elide_redundant_ldweights(nc)
    else:
        nc._n_elided = 0
    nc.compile()
    _cache[key] = nc
    return nc


def _prep_inputs(x, group_by, W, b, C=C_DEFAULT, dt=DT_DEFAULT,
                 layout=LAYOUT_DEFAULT):
    if dt == "f32r":
        np_dt = np.float32
    elif dt == "f16":
        np_dt = np.float16
    else:
        import ml_dtypes
        np_dt = ml_dtypes.bfloat16
    x_flat = np.ascontiguousarray(np.asarray(x, dtype=np.float32)).reshape(B * S, DIN)
    gb = np.asarray(group_by).reshape(B * S)
    W = np.asarray(W, dtype=np.float32)
    b = np.asarray(b, dtype=np.float32)

    idxs, in_maps = [], []
    for g in range(G):
        idx = np.nonzero(gb == g)[0]
        n = min(len(idx), C)
        xt = np.zeros((DIN, C), dtype=np_dt)
        xt[:, :n] = x_flat[idx[:n]].T.astype(np_dt)
        wtT = W[g].reshape(DOUT, DIN).T.astype(np_dt)  # [DIN, DOUT]
        if layout == "ws":
            # [OB, P, KC, P]: wt[ob, p, k, o'] = W^T[k*128+p, ob*128+o'] —
            # the SBUF-resident layout, so each o-block DMA is contiguous.
            wt = np.ascontiguousarray(
                wtT.reshape(KC, P, OB, P).transpose(2, 1, 0, 3))
            bias = np.ascontiguousarray(b[g].reshape(OB, P).T)  # [P, OB]
        else:
            wt = np.ascontiguousarray(wtT)
            bias = np.ascontiguousarray(np.broadcast_to(b[g], (P, DOUT)))
        in_maps.append({"xt": xt, "wt": wt, "bias": bias})
        idxs.append(idx)
    return x_flat, idxs, in_maps, W, b


def _scatter(results, x_flat, idxs, W, b, C=C_DEFAULT, layout=LAYOUT_DEFAULT):
    out_flat = np.empty((B * S, DOUT), dtype=np.float32)
    for g in range(G):
        idx = idxs[g]
        n = min(len(idx), C)
        yv = results[g]["y"]
        if layout == "ws":  # device wrote [DOUT, C]
            out_flat[idx[:n]] = yv[:, :n].T.astype(np.float32)
        else:
            out_flat[idx[:n]] = yv[:n].astype(np.float32)
        if len(idx) > C:  # capacity spill: finish the stragglers on host
            extra = idx[C:]
            out_flat[extra] = x_flat[extra] @ W[g].reshape(DOUT, DIN).T + b[g]
    return out_flat.reshape(B, S, DOUT)


def kernel(x, group_by, W, b):
    nc = _build()
    x_flat, idxs, in_maps, W, b = _prep_inputs(x, group_by, W, b)
    res = run_bass_kernel_spmd(nc, in_maps, list(range(G)))
    return _scatter(res.results, x_flat, idxs, W, b)
